# revision 19
# baseline (speedup 1.0000x reference)
"""Multi-head attention TRN2 Bass kernel (8 NeuronCores, tensor-parallel).

Sharding: Megatron-style TP over (batch x head-group). 8 cores = 2 batches x 4
head-groups of 4 heads each. Each core computes its heads' Q/K/V projections,
masked-softmax attention, and a partial output projection; the host sums the 4
partials per batch (the TP unshard).

v3 design (single fused pipeline, fp16 compute):
  - DMA: few big consolidated transfers (HWDGE trigger cost is fixed ~625ns,
    and one trigger's packets fan out across all 16 DMA engines), spread over
    the three DGE-capable queues (Sync, Activation during lead-in, GpSimd).
  - One global step machine over (mh, h, nt): scores (PE, K=64 partition-
    offset, no zero-pad), exp (ACT, scale=1/8, PSUM->SBUF fp16), mask multiply
    (DVE, nt-paired 2048-wide), ctx (PE, lhsT=[vw_h|1], lag LAG steps).
  - v-projection (per-nt), q-mh1 projection and out-proj(mh0) injected as
    small side quanta between steps to keep PE dense (HAM stays warm).
  - Normalize per head: pctx drained by DVE+GpSimd halves + DVE sums row,
    ones-broadcast MM into utility PSUM, reciprocal_approx_fast, one multiply.
  - PSUM: scores 2x[128,1024] (4 banks) + pctx [65,1024] (2) + util 2x[128,512]
    (2) = 8 banks; lead-in projection pool closes before attention opens.
"""
import os
import sys

for p in ("/opt/trn_rl_repo",):
    if p not in sys.path:
        sys.path.insert(0, p)

from contextlib import ExitStack

import numpy as np

import concourse.bass as bass
import concourse.tile as tile
from concourse import bacc, mybir
from concourse.bass_utils import run_bass_kernel_spmd

F32 = mybir.dt.float32
F32R = mybir.dt.float32r
F16 = mybir.dt.float16
EXP = mybir.ActivationFunctionType.Exp
ts = bass.ts

B, M, N, E = 2, 2048, 2048, 1024  # batch, q-len, k-len, d_model
H, DK = 16, 64                    # heads, head dim
NCORES = 8
GROUPS = 4                        # head groups (cores per batch)
DLOC = (H // GROUPS) * DK         # 256 per-core projection width
HL = H // GROUPS                  # 4 local heads
ET = E // 128                     # 8 k-tiles of the projection contraction
NT = N // 128                    # 16 n-tiles
VSTR = HL * (DK + 1)              # 260: vw slot stride per n-tile
MH = 2                            # m halves of 1024

LAG = int(os.environ.get("K_LAG", "6"))          # ctx lag behind scores


def build_program() -> bass.Bass:
    nc = bacc.Bacc()

    qT_d = nc.dram_tensor("qT", [E, M], F16, kind="ExternalInput")
    kT_d = nc.dram_tensor("kT", [E, N], F16, kind="ExternalInput")
    vT_d = nc.dram_tensor("vT", [E, N], F16, kind="ExternalInput")
    keepT_d = nc.dram_tensor("keepT", [N, M], mybir.dt.uint8, kind="ExternalInput")
    wqT_d = nc.dram_tensor("wqT", [E, DLOC], F16, kind="ExternalInput")
    wkT_d = nc.dram_tensor("wkT", [E, DLOC], F16, kind="ExternalInput")
    wvT_d = nc.dram_tensor("wvT", [E, DLOC], F16, kind="ExternalInput")
    woT_d = nc.dram_tensor("woT", [DLOC, E], F16, kind="ExternalInput")
    out_d = nc.dram_tensor("out", [M, E], F16, kind="ExternalOutput")

    # partition-major views for consolidated DMA
    kT3 = kT_d[:, :].rearrange("(t p) n -> p t n", p=128)
    qT3 = qT_d[:, :].rearrange("(t p) n -> p t n", p=128)
    vT3 = vT_d[:, :].rearrange("(t p) n -> p t n", p=128)
    keep3 = keepT_d[:, :].rearrange("(t p) m -> p t m", p=128)
    wq3 = wqT_d[:, :].rearrange("(t p) d -> p t d", p=128)
    wk3 = wkT_d[:, :].rearrange("(t p) d -> p t d", p=128)
    wv3 = wvT_d[:, :].rearrange("(t p) d -> p t d", p=128)
    wo3 = woT_d[:, :].rearrange("(t p) d -> p t d", p=128)

    with tile.TileContext(nc) as tc, ExitStack() as ctx:
        const_pool = ctx.enter_context(tc.tile_pool(name="const", bufs=1))
        w_pool = ctx.enter_context(tc.tile_pool(name="weights", bufs=1))
        act_pool = ctx.enter_context(tc.tile_pool(name="acts", bufs=1))

        ones64 = const_pool.tile([1, 64], F16)
        nc.vector.memset(ones64[:], 1.0)
        ones64f = const_pool.tile([1, 64], F32)
        nc.vector.memset(ones64f[:], 1.0)
        warm_exp = const_pool.tile([1, 64], F16)
        nc.scalar.activation(warm_exp[:], ones64[:], EXP, scale=0.125)

        wq_sb = w_pool.tile([128, ET * DLOC], F16, tag="wq")
        wk_sb = w_pool.tile([128, ET * DLOC], F16, tag="wk")
        wv_sb = w_pool.tile([128, ET * DLOC], F16, tag="wv")
        wo_sb = w_pool.tile([128, 2 * E], F16, tag="wo")

        # qw/kw: [d, m] / [d, n]; d2 indexes the two 128-row halves of DLOC;
        # within a half, rows 0-63 are the even head, 64-127 the odd head.
        qw = [act_pool.tile([128, M], F16, tag=f"qw{i}", name=f"qw{i}") for i in range(2)]
        kw = [act_pool.tile([128, N], F16, tag=f"kw{i}", name=f"kw{i}") for i in range(2)]
        vw_sb = act_pool.tile([128, NT * VSTR], F16, tag="vw")
        ctxs = [act_pool.tile([128, M], F16, tag=f"ctx{i}", name=f"ctx{i}") for i in range(2)]
        vw3 = vw_sb.rearrange("p (s x) -> p s x", x=DK + 1)
        nc.vector.memset(vw3[:, :, DK : DK + 1], 1.0)  # just the ones columns

        # long-lived pools
        util_ps = ctx.enter_context(tc.tile_pool(name="util_ps", bufs=2, space="PSUM"))
        xq_pool = ctx.enter_context(tc.tile_pool(name="xq", bufs=3))
        vT_pool = ctx.enter_context(tc.tile_pool(name="vTp", bufs=1))
        keep_pool = ctx.enter_context(tc.tile_pool(name="keep", bufs=5))
        au_pool = ctx.enter_context(tc.tile_pool(name="au", bufs=2))
        am_pool = ctx.enter_context(tc.tile_pool(name="am", bufs=8))
        csb_pool = ctx.enter_context(tc.tile_pool(name="csb", bufs=1))
        rbs_pool = ctx.enter_context(tc.tile_pool(name="rbs", bufs=1))
        osb_pool = ctx.enter_context(tc.tile_pool(name="osb", bufs=2))

        # ---- lead-in DMA: consolidated, spread over Sync/ACT/GpSimd ----
        # kT "halves" are n-column halves: kw cols nt 0-7 only need half0.
        xk = xq_pool.tile([128, ET * 1024], F16, tag="xq", name="xk")
        xq0 = xq_pool.tile([128, ET * 1024], F16, tag="xq", name="xq0")
        xk1 = xq_pool.tile([128, ET * 1024], F16, tag="xq", name="xk1")
        vt = vT_pool.tile([128, ET * 2048], F16, tag="vT", name="vt")
        # priority class A first on every queue, then B; each tensor's halves
        # go to both hwdge queues so the critical path gets 2/3 of bandwidth.
        nc.sync.dma_start(wk_sb[:].rearrange("p (t d) -> p t d", t=ET), wk3)
        nc.scalar.dma_start(wq_sb[:].rearrange("p (t d) -> p t d", t=ET), wq3)
        for g, eng in ((0, nc.sync), (1, nc.scalar)):
            eng.dma_start(
                xk[:, g * 4096 : (g + 1) * 4096],
                kT3[:, g * 4 : (g + 1) * 4, 0:1024],
            )
        for g, eng in ((0, nc.sync), (1, nc.scalar)):
            eng.dma_start(
                xq0[:, g * 4096 : (g + 1) * 4096],
                qT3[:, g * 4 : (g + 1) * 4, 0:1024],
            )
        vt3v = vt[:].rearrange("p (t n) -> p t n", t=ET)
        for g, eng in ((0, nc.sync), (1, nc.scalar)):
            eng.dma_start(
                vt3v[:, g * 4 : (g + 1) * 4, 0:1024],
                vT3[:, g * 4 : (g + 1) * 4, 0:1024],
            )
        for g, eng in ((0, nc.sync), (1, nc.scalar)):
            eng.dma_start(
                xk1[:, g * 4096 : (g + 1) * 4096],
                kT3[:, g * 4 : (g + 1) * 4, 1024:2048],
            )
        for g, eng in ((0, nc.sync), (1, nc.scalar)):
            eng.dma_start(
                vt3v[:, g * 4 : (g + 1) * 4, 1024:2048],
                vT3[:, g * 4 : (g + 1) * 4, 1024:2048],
            )

        # ---- lead-in projections: k (both halves) + q (mh0) ----
        with tc.tile_pool(name="lead_ps", bufs=4, space="PSUM") as lead_ps:
            def proj_qk_unit(w_sb, dst, chalf, xt):
                """One 1024-col half of a q/k projection: 32 MMs + 4 copies."""
                pss = []
                for j in range(4):
                    ps = lead_ps.tile([128, 512], F32, tag="lp", name=f"lp{j}")
                    pss.append(ps)
                for et in range(ET):
                    for d2 in range(2):
                        for c2 in range(2):
                            nc.tensor.matmul(
                                pss[d2 * 2 + c2][:],
                                w_sb[:, et * DLOC + d2 * 128 : et * DLOC + (d2 + 1) * 128],
                                xt[:, et * 1024 + c2 * 512 : et * 1024 + (c2 + 1) * 512],
                                start=(et == 0), stop=(et == ET - 1),
                            )
                for j, ps in enumerate(pss):
                    d2, c2 = divmod(j, 2)
                    nc.vector.tensor_copy(
                        dst[d2][:, chalf * 1024 + c2 * 512 : chalf * 1024 + (c2 + 1) * 512],
                        ps[:],
                    )

            proj_qk_unit(wk_sb, kw, 0, xk)
            proj_qk_unit(wq_sb, qw, 0, xq0)

        # keep mask: uint8 in DRAM, gpsimd cast-DMA to fp16 quarter tiles
        keep_tiles = {}

        def dma_keep(mh, qr):
            kt = keep_pool.tile([128, 4 * 1024], F16, tag="keep", name=f"keep{mh}{qr}")
            nc.gpsimd.dma_start(
                kt[:], keep3[:, qr * 4 : (qr + 1) * 4, mh * 1024 : (mh + 1) * 1024]
            )
            keep_tiles[(mh, qr)] = kt

        dma_keep(0, 0)
        nc.gpsimd.dma_start(wv_sb[:].rearrange("p (t d) -> p t d", t=ET), wv3)
        for qr in range(1, 4):
            dma_keep(0, qr)

        # ---- side-work micro-quanta (emitted between attention steps) ----
        # kept small (<=8 MMs) so the PE never idles long enough for the HAM
        # clock gate to re-throttle.
        kh1_ps = {}

        def kh1_micro(j):
            """k half1 projection in 4 micros: (d2, et-half); 8 MMs each."""
            def emit():
                d2, eth = divmod(j, 2)
                if eth == 0:
                    for c2 in range(2):
                        kh1_ps[(d2, c2)] = util_ps.tile(
                            [128, 512], F32, tag="u", name=f"k1p{c2}"
                        )
                for et in range(eth * 4, eth * 4 + 4):
                    for c2 in range(2):
                        nc.tensor.matmul(
                            kh1_ps[(d2, c2)][:],
                            wk_sb[:, et * DLOC + d2 * 128 : et * DLOC + (d2 + 1) * 128],
                            xk1[:, et * 1024 + c2 * 512 : et * 1024 + (c2 + 1) * 512],
                            start=(et == 0), stop=(et == ET - 1),
                        )
                if eth == 1:
                    for c2 in range(2):
                        nc.vector.tensor_copy(
                            kw[d2][:, 1024 + c2 * 512 : 1024 + (c2 + 1) * 512],
                            kh1_ps.pop((d2, c2))[:],
                        )
            return emit

        v_ps = {}

        def v_micro(nt, half):
            def emit():
                if half == 0:
                    v_ps[nt] = util_ps.tile([128, 512], F32, tag="u", name="vp")
                ps = v_ps[nt]
                for et in range(half * 4, half * 4 + 4):
                    nc.tensor.matmul(
                        ps[:, 0:DLOC],
                        vt[:, et * 2048 + nt * 128 : et * 2048 + (nt + 1) * 128],
                        wv_sb[:, ts(et, DLOC)],
                        start=(et == 0), stop=(et == ET - 1),
                    )
                if half == 1:
                    del v_ps[nt]
                    nc.vector.tensor_copy(
                        vw3[:, nt * HL : (nt + 1) * HL, 0:DK],
                        ps[:, 0:DLOC].rearrange("p (s x) -> p s x", x=DK),
                    )
            return emit

        xq1 = [None]

        def q1_dma():
            xq1[0] = xq_pool.tile([128, ET * 1024], F16, tag="xq", name="xq1")
            for g in range(2):
                nc.sync.dma_start(
                    xq1[0][:, g * 4096 : (g + 1) * 4096],
                    qT3[:, g * 4 : (g + 1) * 4, 1024:2048],
                )

        q1_ps = {}

        def q1_micro(d2, c2, half):
            def emit():
                if half == 0:
                    q1_ps[(d2, c2)] = util_ps.tile([128, 512], F32, tag="u", name="qp")
                ps = q1_ps[(d2, c2)]
                for et in range(half * 4, half * 4 + 4):
                    nc.tensor.matmul(
                        ps[:],
                        wq_sb[:, et * DLOC + d2 * 128 : et * DLOC + (d2 + 1) * 128],
                        xq1[0][:, et * 1024 + c2 * 512 : et * 1024 + (c2 + 1) * 512],
                        start=(et == 0), stop=(et == ET - 1),
                    )
                if half == 1:
                    del q1_ps[(d2, c2)]
                    nc.vector.tensor_copy(
                        qw[d2][:, 1024 + c2 * 512 : 1024 + (c2 + 1) * 512], ps[:]
                    )
            return emit

        def oproj_mt(mt, psum_tile_fn):
            """Out-proj for one m-tile: 4 MMs, split copies, 1 DMA trigger."""
            ob = osb_pool.tile([128, 1024], F16, tag="ob", name="ob")
            for ec in range(2):
                po = psum_tile_fn()
                for kt2 in range(2):
                    nc.tensor.matmul(
                        po[:],
                        ctxs[kt2][:, ts(mt, 128)],
                        wo_sb[:, kt2 * E + ec * 512 : kt2 * E + (ec + 1) * 512],
                        start=(kt2 == 0), stop=(kt2 == 1),
                    )
                if ec == 0:
                    nc.vector.tensor_copy(ob[:, ts(ec, 512)], po[:])
                else:
                    nc.scalar.copy(ob[:, ts(ec, 512)], po[:])
            nc.gpsimd.dma_start(out_d[ts(mt, 128), :], ob[:])

        def util_po():
            return util_ps.tile([128, 512], F32, tag="u", name="po")

        def oproj_micro(mt):
            def emit():
                if mt == 0:
                    nc.gpsimd.dma_start(wo_sb[:].rearrange("p (t d) -> p t d", t=2), wo3)
                oproj_mt(mt, util_po)
            return emit

        # schedule
        injections = {}
        # k half1 projection micros land right before scores(nt=8) needs it
        for j in range(4):
            injections.setdefault(5 + j // 2, []).append(kh1_micro(j))
        # v projection: two micros per nt, landing just before ctx(h0, nt)
        for nt in range(NT):
            injections.setdefault(nt + 4, []).append(v_micro(nt, 0))
            injections.setdefault(nt + 5, []).append(v_micro(nt, 1))
        injections.setdefault(16, []).append(q1_dma)
        for j, (d2, c2) in enumerate([(0, 0), (0, 1), (1, 0), (1, 1)]):
            injections.setdefault(20 + 2 * j, []).append(q1_micro(d2, c2, 0))
            injections.setdefault(21 + 2 * j, []).append(q1_micro(d2, c2, 1))
        # out-proj mh0 during mh1, preferentially at head-boundary steps to
        # fill the pctx-drain bubble; mh0-h3 normalize is emitted at 63+LAG.
        for i, st in enumerate((70, 71, 80, 81, 96, 97, 112, 113)):
            injections.setdefault(st, []).append(oproj_micro(i))
        prefetches = {24: (1, 0), 52: (1, 1), 56: (1, 2), 60: (1, 3)}

        s_ps = ctx.enter_context(tc.tile_pool(name="s_ps", bufs=2, space="PSUM"))
        c_ps = ctx.enter_context(tc.tile_pool(name="c_ps", bufs=1, space="PSUM"))

        am_half = {}   # step -> (am_pair, half_idx)
        au_cur = [None]
        pctx_cur = [None]

        def head_of(s):
            mh, r = divmod(s, 64)
            return mh, r // 16, r % 16

        def emit_scores_exp_mask(s):
            mh, h, nt = head_of(s)
            d2, hl = divmod(h, 2)
            base = hl * 64
            ps = s_ps.tile([128, 1024], F32, tag="ps", name="ps")
            for c2 in range(2):
                nc.tensor.matmul(
                    ps[:, ts(c2, 512)],
                    kw[d2][base : base + 64, ts(nt, 128)],
                    qw[d2][base : base + 64, mh * 1024 + c2 * 512 : mh * 1024 + (c2 + 1) * 512],
                    start=True, stop=True,
                )
            au = au_pool.tile([128, 1024], F16, tag="au", name="au")
            nc.scalar.activation(au[:], ps[:], EXP, scale=0.125)
            am = am_pool.tile([128, 1024], F16, tag="am", name="am")
            kt = keep_tiles[(mh, nt // 4)]
            nc.vector.tensor_mul(am[:], au[:], kt[:, ts(nt % 4, 1024)])
            am_half[s] = am

        def emit_normalize(mh, h, pctx):
            d2, hl = divmod(h, 2)
            base = hl * 64
            csb = csb_pool.tile([64, 1024], F32, tag="csb", name="csb")
            sums = csb_pool.tile([1, 1024], F16, tag="sums", name="sums")
            # parallel drain of pctx: DVE + ACT halves, sums row on ACT
            nc.vector.tensor_copy(csb[:, 0:512], pctx[0:64, 0:512])
            nc.scalar.copy(csb[:, 512:1024], pctx[0:64, 512:1024])
            nc.scalar.copy(sums[:], pctx[64:65, :])
            rbs = rbs_pool.tile([64, 1024], F32, tag="rbs", name="rbs")
            for c2 in range(2):
                prb = util_ps.tile([128, 512], F32, tag="u", name="prb")
                nc.tensor.matmul(
                    prb[0:64, :], ones64[:], sums[:, ts(c2, 512)],
                    start=True, stop=True,
                )
                nc.vector.reciprocal_approx_fast(rbs[:, ts(c2, 512)], prb[0:64, :])
            nc.vector.tensor_mul(
                ctxs[d2][base : base + 64, mh * 1024 : (mh + 1) * 1024],
                csb[:, :],
                rbs[:],
            )

        def emit_ctx(s):
            mh, h, nt = head_of(s)
            if nt == 0:
                pctx_cur[0] = c_ps.tile([65, 1024], F32, tag="pctx", name="pctx")
            am = am_half.pop(s)
            pctx = pctx_cur[0]
            for c2 in range(2):
                nc.tensor.matmul(
                    pctx[:, ts(c2, 512)],
                    vw_sb[:, nt * VSTR + h * 65 : nt * VSTR + (h + 1) * 65],
                    am[:, ts(c2, 512)],
                    start=(nt == 0), stop=(nt == NT - 1),
                )
            if nt == NT - 1:
                emit_normalize(mh, h, pctx)

        for s in range(128 + LAG):
            if s in prefetches:
                dma_keep(*prefetches[s])
            for qta in injections.get(s, ()):
                qta()
            if s < 128:
                emit_scores_exp_mask(s)
            if s >= LAG:
                emit_ctx(s - LAG)

        # tail: out-proj mh1, pipelined over the freed scores PSUM + util
        def sps_po():
            return s_ps.tile([128, 512], F32, tag="ps", name="pot",
                             padded_shape=[128, 1024])

        for i in range(8):
            mt = 8 + i
            oproj_mt(mt, sps_po if i % 2 == 0 else util_po)

    nc.finalize()
    return nc


_PROGRAM = None


def _get_program():
    global _PROGRAM
    if _PROGRAM is None:
        _PROGRAM = build_program()
    return _PROGRAM


def _make_in_maps(q, k, v, mask, Wq, Wk, Wv, Wo):
    q = np.asarray(q, dtype=np.float32)
    k = np.asarray(k, dtype=np.float32)
    v = np.asarray(v, dtype=np.float32)
    mask = np.asarray(mask)
    Wq = np.asarray(Wq, dtype=np.float32)
    Wk = np.asarray(Wk, dtype=np.float32)
    Wv = np.asarray(Wv, dtype=np.float32)
    Wo = np.asarray(Wo, dtype=np.float32)

    per_batch = {}
    for b in range(B):
        per_batch[b] = dict(
            qT=np.ascontiguousarray(q[b].T.astype(np.float16)),
            kT=np.ascontiguousarray(k[b].T.astype(np.float16)),
            vT=np.ascontiguousarray(v[b].T.astype(np.float16)),
            keepT=np.ascontiguousarray(
                np.logical_not(mask[b]).T.astype(np.uint8)
            ),
        )

    in_maps = []
    for c in range(NCORES):
        b, hg = divmod(c, GROUPS)
        sl = slice(hg * DLOC, (hg + 1) * DLOC)
        in_maps.append(
            dict(
                per_batch[b],
                wqT=np.ascontiguousarray(Wq[sl].T.astype(np.float16)),
                wkT=np.ascontiguousarray(Wk[sl].T.astype(np.float16)),
                wvT=np.ascontiguousarray(Wv[sl].T.astype(np.float16)),
                woT=np.ascontiguousarray(Wo[:, sl].T.astype(np.float16)),
            )
        )
    return in_maps


def _run(in_maps, trace=False):
    nc = _get_program()
    return run_bass_kernel_spmd(
        nc, in_maps, list(range(NCORES)), trace=trace
    )


def _assemble(results):
    out = np.zeros((B, M, E), dtype=np.float32)
    for c in range(NCORES):
        b = c // GROUPS
        out[b] += results[c]["out"].astype(np.float32)
    return out


def kernel(q, k, v, mask, Wq, Wk, Wv, Wo):
    in_maps = _make_in_maps(q, k, v, mask, Wq, Wk, Wv, Wo)
    res = _run(in_maps, trace=False)
    return _assemble(res.results)


def run_profiled(q, k, v, mask, Wq, Wk, Wv, Wo):
    """Like kernel(), but traces execution; returns (out, BassKernelResults)."""
    in_maps = _make_in_maps(q, k, v, mask, Wq, Wk, Wv, Wo)
    res = _run(in_maps, trace=True)
    return _assemble(res.results), res
